# revision 56
# baseline (speedup 1.0000x reference)
"""Trainium2 Bass kernel: GarmentPersonCrossAttention (B=4, N=2048, M=1024,
DQ=1024, DC=768, H=16, DH=64), distributed over 8 NeuronCores.

Sharding: core i handles batch i//2 and person-row half i%2 (1024 rows).
Everything is local per core (garment-side stats + K/V projections are
recomputed by both cores of a batch pair) -- no collectives.

Host-side algebraic folds (exact linear algebra, numpy):
  - LN affine (gamma, beta) folded into Wq/Wk/Wv plus bias rows.
  - softmax scale DH**-0.5 folded into Wq (and its bias).
  - K-projection bias dropped entirely: it shifts every score of a given
    (n, h) row by the same constant, which softmax cancels.
  - V-projection bias folded into the output bias: attention weights sum
    to 1, so att = att_nobias + bv; (att+bv)@Wo+bo = att@Wo + (bv@Wo+bo).
  - concat([residual, att]) @ Wf + bf
        = residual @ Wf[:DQ] + att @ (Wo @ Wf[DQ:]) + bias
    so Wo and the bottom half of Wf collapse into one matrix WoF.

LayerNorm without row-major passes: the raw inputs are DMA-transposed to
feature-major at time 0; row means and second moments then come from
ones-vector matmuls over those transposed tiles (stationary [D,1] =
1/D), landing free-indexed in PSUM exactly as needed for broadcasting.
mu/rstd rows are broadcast across partitions with stride-0 SBUF->SBUF
DMAs and the tiles are normalized in place-adjacent copies on DVE:
z = (x + (-mu)) * rstd. The raw person transposes also serve as the
phase-D residual operand.

DMA packet discipline: every weight is loaded as full [128, row] tiles
(2KB per partition per descriptor) and sliced along the free dim; all
small bounces (stats, reciprocals) stay SBUF->SBUF. The DMA engines are
packet-rate-bound (~60ns/packet), so packet count, not bytes, is the
budget.

Device pipeline per core:
  B (bf16 matmuls, fp32 PSUM): kT = Wk'.T @ zgT to fp8e4, DMA-remapped
     into the DoubleRow layout k8[h] [32, 2, m] (contraction pairs
     d=(p, 32+p)); v = zgT.T @ Wv' to fp8e4 in paired layout vt_pair[t]
     [128, H, 2, 128] = [ones | zeros | v] (dim2 = m-tile parity; the
     ones column makes the av matmul emit softmax denominators in pa
     row 0); qT likewise (bias added on ACT) to q8[h].
  C (fp8e4 DoubleRow matmuls; 2x row throughput, and the av matmul also
     recovers the 65/128-partition waste of the bf16 layout): per (head,
     n-window 512): 8 score matmuls (256 PE cycles each), exp on ACT
     with bias -4 (softmax-invariant shift keeping exp in fp8e4 range;
     max score ~8.2 on these inputs), fp8 exps feed the paired av matmul
     accumulating attT over 4 m-tile-pairs. Denominator reciprocal via
     DVE reciprocal_approx_fast on pa row 0 (the custom op drops nonzero
     partition offsets on HW, hence the ones-first sub layout), then a
     stride-0 SBUF broadcast and a DVE multiply during PSUM evacuation.
     The chunk is software-pipelined (scores+exp of pair t+1 before av
     of t) and B-q / phase-D matmul work is drip-fed between its stages
     through a filler queue so the in-order PE queue never stalls on exp
     latency.
  D (bf16): out[n,dq] = xT.T @ Wf_top + attT.T @ WoF + bias, fused
     tensor_add evacuation (fp32 out).
"""

import os
import sys

import numpy as np

for _p in ("/opt/trn_rl_repo",):
    if _p not in sys.path and os.path.isdir(_p):
        sys.path.append(_p)

import ml_dtypes

# Problem constants (hardcoded per contest rules).
B, N, M = 4, 2048, 1024
DQ, DC = 1024, 768
H, DH = 16, 64
INNER = H * DH
SCALE = DH ** -0.5
EPS = 1e-5
ESHIFT = 4.0          # exp(s - ESHIFT): cancels in softmax, keeps fp8 range
NCORES = 8
NPC = N // 2          # person rows per core
P = 128               # partitions
NT = NPC // P         # 8 person row tiles per core
MT = M // P           # 8 garment row tiles
KQ = DQ // P          # 8 contraction tiles for person features
KC = DC // P          # 6 contraction tiles for garment features
KI = INNER // P       # 8 inner tiles
NW = NPC // 512       # 2 n-windows per core
DHP = 96              # av-stationary sub width: [ones | zeros(31) | v(64)]

_CACHE = {}


def _build_nc():
    import concourse.bass as bass
    import concourse.tile as tile
    from concourse import bacc, mybir
    from contextlib import ExitStack
    from collections import deque

    f32 = mybir.dt.float32
    bf16 = mybir.dt.bfloat16
    fp8 = mybir.dt.float8e4
    AF = mybir.ActivationFunctionType
    ALU = mybir.AluOpType
    DR = mybir.MatmulPerfMode.DoubleRow

    nc = bacc.Bacc("TRN2", target_bir_lowering=False, debug=False)

    # ---- DRAM parameters (per-core shards; weights replicated) ----
    xp = nc.dram_tensor("xp", [NPC, DQ], bf16, kind="ExternalInput").ap()
    xg = nc.dram_tensor("xg", [M, DC], bf16, kind="ExternalInput").ap()
    # wq/wk host-rearranged to [it, p, kt, 128] so a per-it stationary
    # tile loads with one long contiguous run per partition.
    wqr = nc.dram_tensor("wqr", [KI, P, KQ, P], bf16, kind="ExternalInput").ap()
    wkr = nc.dram_tensor("wkr", [KI, P, KC, P], bf16, kind="ExternalInput").ap()
    wv = nc.dram_tensor("wv", [DC, INNER], bf16, kind="ExternalInput").ap()
    wof = nc.dram_tensor("wof", [INNER, DQ], bf16, kind="ExternalInput").ap()
    wft = nc.dram_tensor("wft", [DQ, DQ], bf16, kind="ExternalInput").ap()
    bq = nc.dram_tensor("bq", [INNER], f32, kind="ExternalInput").ap()
    bout = nc.dram_tensor("bout", [DQ], f32, kind="ExternalInput").ap()
    out = nc.dram_tensor("out", [NPC, DQ], f32, kind="ExternalOutput").ap()

    # Internal DRAM bounce rows (stride-0 partition broadcasts must source
    # from DRAM).
    rb_d = nc.dram_tensor("recip_scratch", [H * NW, 512], bf16).ap()
    st_d = nc.dram_tensor("stat_scratch", [4, NPC], bf16).ap()

    with tile.TileContext(nc) as tc, ExitStack() as ctx:
        const = ctx.enter_context(tc.tile_pool(name="const", bufs=1, side="left"))
        small = ctx.enter_context(tc.tile_pool(name="small", bufs=4, side="left"))
        tmp_pool = ctx.enter_context(tc.tile_pool(name="tmp", bufs=2, side="left"))

        # ---- constants ----
        eps_t = const.tile([P, 1], f32, name="eps_t")
        nc.vector.memset(eps_t, EPS)
        nshift_t = const.tile([P, 1], f32, name="nshift_t")
        nc.vector.memset(nshift_t, -ESHIFT)
        mone_t = const.tile([P, 1], f32, name="mone_t")
        nc.vector.memset(mone_t, -1.0)
        oneg_t = const.tile([P, 1], bf16, name="oneg_t")
        nc.vector.memset(oneg_t, 1.0 / DC)
        onep_t = const.tile([P, 1], bf16, name="onep_t")
        nc.vector.memset(onep_t, 1.0 / DQ)
        bq_sb = const.tile([P, KI], f32, name="bq_sb")
        nc.sync.dma_start(out=bq_sb, in_=bq.rearrange("(t p) -> p t", p=P))
        bout_bc = const.tile([P, DQ], f32, name="bout_bc")
        nc.sync.dma_start(
            out=bout_bc,
            in_=bass.AP(tensor=bout.tensor, offset=bout.offset, ap=[[0, P], [1, DQ]]),
        )

        # Persistent SBUF tiles (left stack).
        # Matmul operand base partitions must be in {0, 32, 64}: pack 3
        # heads per 128-partition tile (partitions 96-127 unused).
        NG = (H + 2) // 3
        q8_pool = ctx.enter_context(tc.tile_pool(name="q8", bufs=NG, side="left"))
        k8_pool = ctx.enter_context(tc.tile_pool(name="k8", bufs=NG, side="left"))
        q8p = [q8_pool.tile([P, 2, NPC], fp8, name=f"q8_{g}", tag="q8")
               for g in range(NG)]
        k8p = [k8_pool.tile([P, 2, M], fp8, name=f"k8_{g}", tag="k8")
               for g in range(NG)]

        def q8(h):
            r = 32 * (h % 3)
            return q8p[h // 3][r:r + 32, :, :]

        def k8(h):
            r = 32 * (h % 3)
            return k8p[h // 3][r:r + 32, :, :]

        v_pool = ctx.enter_context(tc.tile_pool(name="vsb", bufs=MT // 2, side="left"))
        vt_pair = [v_pool.tile([P, H, 2, DHP], fp8, name=f"v{t}", tag="v")
                   for t in range(MT // 2)]
        att_pool = ctx.enter_context(tc.tile_pool(name="att", bufs=KI, side="left"))
        att = [att_pool.tile([P, NPC], bf16, name=f"att{i}", tag="att")
               for i in range(KI)]
        xptr_pool = ctx.enter_context(tc.tile_pool(name="xptr", bufs=KQ, side="left"))
        wout_pool = ctx.enter_context(tc.tile_pool(name="wout", bufs=16, side="left"))
        ex_pool = ctx.enter_context(tc.tile_pool(name="exp", bufs=3, side="left"))
        bcst_pool = ctx.enter_context(tc.tile_pool(name="bcst", bufs=1, side="left"))
        zpt_pool = ctx.enter_context(tc.tile_pool(name="zpt", bufs=KQ, side="left"))

        # ---- raw-input feature-major transposes: start at time 0 ----
        xgt_ctx = ExitStack()
        xgt_pool = xgt_ctx.enter_context(
            tc.tile_pool(name="xgt", bufs=KC, side="right")
        )
        xgt = [xgt_pool.tile([P, M], bf16, name=f"xgt{j}", tag="xgt")
               for j in range(KC)]
        for j in range(KC):
            nc.sync.dma_start_transpose(xgt[j], xg[:, j * P:(j + 1) * P])
        xptr = []
        for kt in range(KQ):
            xr = xptr_pool.tile([P, NPC], bf16, tag="xpt")
            nc.sync.dma_start_transpose(xr, xp[:, kt * P:(kt + 1) * P])
            xptr.append(xr)

        bpsum_ctx = ExitStack()
        psum = bpsum_ctx.enter_context(
            tc.tile_pool(name="psum", bufs=4, space="PSUM")
        )

        # ---- row stats via ones-matmuls over the transposed tiles ----
        def side_stats(tiles, kk, ones_sc, nrows, nmu_bc, rstd_bc, srow):
            """mean/E[x^2] per row from [D(part), rows(free)] tiles; fills
            the partition-broadcast (-mu) and rstd tiles [P, nrows] bf16."""
            strow_ctx = ExitStack()
            strow = strow_ctx.enter_context(
                tc.tile_pool(name="strow", bufs=1, side="right")
            )
            for ch in range(nrows // 512):
                pm = psum.tile([1, 512], f32, tag="st")
                for kt in range(kk):
                    nc.tensor.matmul(
                        pm,
                        ones_sc[0:tiles[kt].shape[0], :],
                        tiles[kt][:, ch * 512:(ch + 1) * 512],
                        start=(kt == 0),
                        stop=(kt == kk - 1),
                    )
                pq2 = psum.tile([1, 512], f32, tag="st")
                for kt in range(kk):
                    sq = tmp_pool.tile([P, 512], bf16, tag="sq")
                    nc.vector.tensor_mul(
                        sq,
                        tiles[kt][:, ch * 512:(ch + 1) * 512],
                        tiles[kt][:, ch * 512:(ch + 1) * 512],
                    )
                    nc.tensor.matmul(
                        pq2,
                        ones_sc[0:P, :],
                        sq,
                        start=(kt == 0),
                        stop=(kt == kk - 1),
                    )
                mean_sb = strow.tile([1, 512], f32, tag="mean")
                nc.vector.tensor_copy(mean_sb, pm)
                m2 = strow.tile([1, 512], f32, tag="m2")
                nc.vector.tensor_mul(m2, mean_sb, mean_sb)
                var = strow.tile([1, 512], f32, tag="var")
                nc.vector.tensor_sub(var, pq2, m2)
                std = strow.tile([1, 512], f32, tag="stds")
                nc.scalar.activation(
                    out=std, in_=var, func=AF.Sqrt, bias=eps_t[0:1, :]
                )
                rst = strow.tile([1, 512], f32, tag="rst")
                nc.vector.reciprocal_approx_fast(out=rst, in_=std)
                rstb = strow.tile([1, 512], bf16, tag="rstb")
                nc.vector.tensor_copy(rstb, rst)
                nmub = strow.tile([1, 512], bf16, tag="nmub")
                nc.vector.tensor_scalar(
                    out=nmub, in0=mean_sb, scalar1=mone_t[0:1, :], scalar2=None,
                    op0=ALU.mult,
                )
                for r, (sr, dst) in enumerate(
                    ((nmub, nmu_bc), (rstb, rstd_bc))
                ):
                    off = (srow + r) * NPC + ch * 512
                    nc.sync.dma_start(
                        out=bass.AP(
                            tensor=st_d.tensor, offset=off, ap=[[1, 1], [1, 512]]
                        ),
                        in_=sr,
                    )
                    nc.sync.dma_start(
                        out=dst[:, ch * 512:(ch + 1) * 512],
                        in_=bass.AP(
                            tensor=st_d.tensor, offset=off, ap=[[0, P], [1, 512]]
                        ),
                    )
            strow_ctx.close()

        gnmu_bc = bcst_pool.tile([P, M], bf16, name="gnmu_bc")
        grstd_bc = bcst_pool.tile([P, M], bf16, name="grstd_bc")
        side_stats(xgt, KC, oneg_t, M, gnmu_bc, grstd_bc, 0)
        # Normalize the garment transposes in place: z = (x - mu) * rstd.
        for kt in range(KC):
            for ch in range(M // 512):
                s = slice(ch * 512, (ch + 1) * 512)
                t0 = tmp_pool.tile([P, 512], bf16, tag="nrm")
                nc.vector.tensor_add(t0, xgt[kt][:, s], gnmu_bc[:, s])
                nc.vector.tensor_mul(xgt[kt][:, s], t0, grstd_bc[:, s])
        zgt = xgt

        # =========== Phase B-k: kT = Wk'.T @ zgT -> fp8 -> k8 remap ========
        with (
            tc.tile_pool(name="wksb", bufs=3, side="right") as wksb,
            tc.tile_pool(name="ktf8", bufs=2, side="right") as ktf8p,
        ):
            for it in range(KI):
                wcol = wksb.tile([P, KC, P], bf16, tag="wk")
                nc.sync.dma_start(out=wcol, in_=wkr[it])
                ktf8 = ktf8p.tile([P, M], fp8, tag="ktf8")
                for mch in range(M // 512):
                    pk = psum.tile([P, 512], f32, tag="ps")
                    for kt in range(KC):
                        nc.tensor.matmul(
                            pk,
                            wcol[:, kt, :],
                            zgt[kt][:, mch * 512:(mch + 1) * 512],
                            start=(kt == 0),
                            stop=(kt == KC - 1),
                        )
                    nc.vector.tensor_copy(ktf8[:, mch * 512:(mch + 1) * 512], pk)
                for hh in range(2):
                    h = 2 * it + hh
                    for j in range(2):
                        nc.sync.dma_start(
                            out=k8(h)[:, j, :],
                            in_=ktf8[64 * hh + 32 * j:64 * hh + 32 * (j + 1), :],
                        )

        # Person stats + normalized zpT (raw xptr stays for the residual).
        pnmu_bc = bcst_pool.tile([P, NPC], bf16, name="pnmu_bc")
        prstd_bc = bcst_pool.tile([P, NPC], bf16, name="prstd_bc")
        side_stats(xptr, KQ, onep_t, NPC, pnmu_bc, prstd_bc, 2)
        zpt = []
        for kt in range(KQ):
            zt = zpt_pool.tile([P, NPC], bf16, name=f"zpt{kt}", tag="zpt")
            for ch in range(NPC // 512):
                s = slice(ch * 512, (ch + 1) * 512)
                t0 = tmp_pool.tile([P, 512], bf16, tag="nrm")
                nc.vector.tensor_add(t0, xptr[kt][:, s], pnmu_bc[:, s])
                nc.vector.tensor_mul(zt[:, s], t0, prstd_bc[:, s])
            zpt.append(zt)

        # ====== Phase B-v: v[m, inner] = zgT.T @ Wv' -> fp8 paired =========
        for g in range(MT // 2):
            # Sub layout [96] = [ones | zeros(31) | v(64)]: denominators
            # land in pa row 0 (reciprocal_approx_fast drops nonzero
            # partition offsets on HW); the numerator sits at rows 32:96 and
            # is evacuated as two 32-partition spans (DVE APs want 32-aligned
            # bases and spans over 32 only from base 0/64).
            nc.vector.memset(vt_pair[g][:, :, :, 0:1], 1.0)
            nc.vector.memset(vt_pair[g][:, :, :, 1:32], 0.0)
        with tc.tile_pool(name="wvp", bufs=KC, side="right") as wvp:
            wv_sb = []
            for kt in range(KC):
                wvc = wvp.tile([P, INNER], bf16, tag="wv")
                nc.sync.dma_start(out=wvc, in_=wv[kt * P:(kt + 1) * P, :])
                wv_sb.append(wvc)
            for mt in range(MT):
                for ich in range(2):
                    pv = psum.tile([P, 512], f32, tag="ps")
                    for kt in range(KC):
                        nc.tensor.matmul(
                            pv,
                            zgt[kt][:, mt * P:(mt + 1) * P],
                            wv_sb[kt][:, ich * 512:(ich + 1) * 512],
                            start=(kt == 0),
                            stop=(kt == KC - 1),
                        )
                    nc.vector.tensor_copy(
                        vt_pair[mt // 2][:, ich * 8:(ich + 1) * 8, mt % 2, 32:32 + DH],
                        pv.rearrange("p (h d) -> p h d", h=8),
                    )
        bpsum_ctx.close()
        xgt_ctx.close()

        # Output-phase weights, as contiguous row-tiles (after xgt freed).
        wof_sb = []
        for it in range(KI):
            wo_t = wout_pool.tile([P, DQ], bf16, tag="wof")
            nc.sync.dma_start(out=wo_t, in_=wof[it * P:(it + 1) * P, :])
            wof_sb.append(wo_t)
        wft_sb = []
        for kt in range(KQ):
            wf_t = wout_pool.tile([P, DQ], bf16, tag="wft")
            nc.sync.dma_start(out=wf_t, in_=wft[kt * P:(kt + 1) * P, :])
            wft_sb.append(wf_t)

        # ====== Phases B-q, C, D: interleaved via a filler queue ============
        # The attention inner loop is ACT-bound (exp); B-q and D matmul work
        # is drip-fed between its stages so the in-order PE queue never
        # stalls on exp latency.
        ps1_pool = ctx.enter_context(tc.tile_pool(name="ps1", bufs=5, space="PSUM"))
        pa_pool = ctx.enter_context(tc.tile_pool(name="pa", bufs=2, space="PSUM"))

        filler = deque()

        def pop_filler(k):
            budget = k
            while budget > 0 and filler:
                try:
                    next(filler[0])
                    budget -= 1
                except StopIteration:
                    filler.popleft()

        def c_chunk(h, w, popk=2):
            """Attention for head h over n-window w (512 cols), fp8 DoubleRow.

            Software-pipelined: scores+exp of m-tile-pair t+1 are emitted
            before the av matmul of pair t, so the PE never stalls on the
            ACT exp latency.
            """
            it_h, row_h = h // 2, (h % 2) * DH
            n0 = w * 512
            qslice = q8(h)[:, :, n0:n0 + 512]
            pa = pa_pool.tile([P, 512], f32, tag="pa")
            NP2 = MT // 2

            def scores_exp(t):
                ex2 = ex_pool.tile([P, 2, 512], fp8, tag="ex")
                for j in range(2):
                    psj = ps1_pool.tile([P, 512], f32, tag="ps")
                    nc.tensor.matmul(
                        psj,
                        k8(h)[:, :, (2 * t + j) * P:(2 * t + j + 1) * P],
                        qslice,
                        perf_mode=DR,
                    )
                    nc.scalar.activation(
                        out=ex2[:, j, :], in_=psj, func=AF.Exp, bias=nshift_t
                    )
                return ex2

            exs = [scores_exp(0)]
            pop_filler(popk)
            for t in range(NP2):
                if t + 1 < NP2:
                    exs.append(scores_exp(t + 1))
                    pop_filler(popk)
                nc.tensor.matmul(
                    pa[0:DHP, :],
                    vt_pair[t][:, h, :, :],
                    exs[t],
                    start=(t == 0),
                    stop=(t == NP2 - 1),
                    perf_mode=DR,
                )
            recip = small.tile([1, 512], f32, tag="recip", bufs=2)
            nc.vector.reciprocal_approx_fast(out=recip, in_=pa[0:1, :])
            recb = small.tile([1, 512], bf16, tag="recb", bufs=2)
            nc.vector.tensor_copy(recb, recip)
            idx = h * NW + w
            nc.sync.dma_start(out=rb_d[idx:idx + 1, :], in_=recb)
            bc = small.tile([DH, 512], bf16, tag="bc", bufs=2)
            nc.sync.dma_start(
                out=bc,
                in_=bass.AP(
                    tensor=rb_d.tensor, offset=idx * 512, ap=[[0, DH], [1, 512]]
                ),
            )
            nc.vector.tensor_mul(
                att[it_h][row_h:row_h + 32, n0:n0 + 512],
                pa[32:64, :],
                bc[0:32, :],
            )
            nc.vector.tensor_mul(
                att[it_h][row_h + 32:row_h + DH, n0:n0 + 512],
                pa[64:96, :],
                bc[32:64, :],
            )
            pop_filler(popk)

        pf_pool = ctx.enter_context(tc.tile_pool(name="pf", bufs=1, space="PSUM"))
        outp = ctx.enter_context(tc.tile_pool(name="outp", bufs=2, side="left"))

        def d_gen(ch, nt):
            """out[n-tile nt, 512-col chunk ch] (bf16 matmuls, fp32 out)."""
            pf = pf_pool.tile([P, 512], f32, tag="pf")
            for kt in range(KQ):
                nc.tensor.matmul(
                    pf,
                    xptr[kt][:, nt * P:(nt + 1) * P],
                    wft_sb[kt][:, ch * 512:(ch + 1) * 512],
                    start=(kt == 0),
                    stop=False,
                )
                if kt % 4 == 3:
                    yield
            for it in range(KI):
                nc.tensor.matmul(
                    pf,
                    att[it][:, nt * P:(nt + 1) * P],
                    wof_sb[it][:, ch * 512:(ch + 1) * 512],
                    start=False,
                    stop=(it == KI - 1),
                )
                if it % 4 == 3:
                    yield
            o_t = outp.tile([P, 512], f32, tag="o")
            nc.vector.tensor_add(o_t, pf, bout_bc[:, ch * 512:(ch + 1) * 512])
            nc.sync.dma_start(
                out=out[nt * P:(nt + 1) * P, ch * 512:(ch + 1) * 512],
                in_=o_t,
            )
            yield

        with (
            tc.tile_pool(name="wqsb", bufs=2, side="right") as wqsb,
            tc.tile_pool(name="qtf8", bufs=2, side="right") as qtf8p,
        ):
            def bq_gen(it):
                wcol = wqsb.tile([P, KQ, P], bf16, tag="wq")
                nc.sync.dma_start(out=wcol, in_=wqr[it])
                qtf8 = qtf8p.tile([P, NPC], fp8, tag="qtf8")
                for nch in range(NPC // 512):
                    pq = pa_pool.tile([P, 512], f32, tag="pa")
                    for kt in range(KQ):
                        nc.tensor.matmul(
                            pq,
                            wcol[:, kt, :],
                            zpt[kt][:, nch * 512:(nch + 1) * 512],
                            start=(kt == 0),
                            stop=(kt == KQ - 1),
                        )
                        if kt % 4 == 3:
                            yield
                    nc.scalar.add(
                        out=qtf8[:, nch * 512:(nch + 1) * 512],
                        in_=pq,
                        add=bq_sb[:, it:it + 1],
                    )
                    yield
                for hh in range(2):
                    h = 2 * it + hh
                    for j in range(2):
                        nc.sync.dma_start(
                            out=q8(h)[:, j, :],
                            in_=qtf8[64 * hh + 32 * j:64 * hh + 32 * (j + 1), :],
                        )
                yield

            # B-q for it=0 runs undiluted (nothing to overlap with yet).
            for _ in bq_gen(0):
                pass
            # C window 0, with B-q for it+1 drip-fed into the chunks.
            for it in range(KI):
                if it + 1 < KI:
                    filler.append(bq_gen(it + 1))
                c_chunk(2 * it, 0)
                c_chunk(2 * it + 1, 0)
            pop_filler(10 ** 9)

        # ====== Phase C window 1 + Phase D window 0 interleaved =============
        for h in range(H):
            if h % 2 == 0:
                filler.append(d_gen(h // 8, (h // 2) % 4))
            c_chunk(h, 1)
        pop_filler(10 ** 9)

        # ====== Phase D window 1 ============================================
        for nt in range(4, NT):
            for ch in range(2):
                for _ in d_gen(ch, nt):
                    pass

    nc.compile()
    return nc


def get_nc():
    if "nc" not in _CACHE:
        _CACHE["nc"] = _build_nc()
    return _CACHE["nc"]


def make_in_maps(inputs):
    """Host-side folding + sharding. Returns one input dict per core."""
    bf = ml_dtypes.bfloat16
    pf_ = np.asarray(inputs["person_features"], np.float32)
    gf_ = np.asarray(inputs["garment_features"], np.float32)
    Wq = np.asarray(inputs["Wq"], np.float32)
    Wk = np.asarray(inputs["Wk"], np.float32)
    Wv = np.asarray(inputs["Wv"], np.float32)
    Wo = np.asarray(inputs["Wo"], np.float32)
    bo = np.asarray(inputs["bo"], np.float32)
    Wf = np.asarray(inputs["Wf"], np.float32)
    bff = np.asarray(inputs["bf"], np.float32)
    gq = np.asarray(inputs["gq"], np.float32)
    betaq = np.asarray(inputs["betaq"], np.float32)
    gk = np.asarray(inputs["gk"], np.float32)
    betak = np.asarray(inputs["betak"], np.float32)

    wq_f = (gq[:, None] * Wq) * np.float32(SCALE)
    bq_f = (betaq @ Wq) * np.float32(SCALE)
    wk_f = gk[:, None] * Wk
    wv_f = gk[:, None] * Wv
    bv_f = betak @ Wv
    wf_top = np.ascontiguousarray(Wf[:DQ])
    wf_bot = Wf[DQ:]
    wof = (Wo.astype(np.float64) @ wf_bot.astype(np.float64)).astype(np.float32)
    # bv folded via softmax-weights-sum-to-1; bk dropped via softmax shift
    # invariance.
    bout = ((bo + bv_f @ Wo) @ wf_bot + bff).astype(np.float32)

    wqr = np.ascontiguousarray(
        wq_f.reshape(KQ, P, KI, P).transpose(2, 1, 0, 3)
    )
    wkr = np.ascontiguousarray(
        wk_f.reshape(KC, P, KI, P).transpose(2, 1, 0, 3)
    )
    shared = {
        "wqr": wqr.astype(bf),
        "wkr": wkr.astype(bf),
        "wv": np.ascontiguousarray(wv_f).astype(bf),
        "wof": wof.astype(bf),
        "wft": wf_top.astype(bf),
        "bq": np.ascontiguousarray(bq_f),
        "bout": bout,
    }
    in_maps = []
    for core in range(NCORES):
        b, half = divmod(core, 2)
        m = dict(shared)
        m["xp"] = np.ascontiguousarray(pf_[b, half * NPC:(half + 1) * NPC]).astype(bf)
        m["xg"] = np.ascontiguousarray(gf_[b]).astype(bf)
        in_maps.append(m)
    return in_maps


def assemble(results):
    out = np.empty((B, N, DQ), np.float32)
    for core in range(NCORES):
        b, half = divmod(core, 2)
        out[b, half * NPC:(half + 1) * NPC] = results[core]["out"]
    return out


def kernel(**inputs):
    from concourse.bass_utils import run_bass_kernel_spmd

    nc = get_nc()
    in_maps = make_in_maps(inputs)
    res = run_bass_kernel_spmd(nc, in_maps, list(range(NCORES)))
    return assemble(res.results)
